# revision 3
# baseline (speedup 1.0000x reference)
"""Trainium2 Bass kernel for nn_MASNET2 (structure-attention warped resampling).

Per batch:
  1. axis-max marginals of structure_att:
       y-profile: DVE free-dim max; x-profile: cc-fold (DVE+Pool) +
       gpsimd partition_all_reduce (profiles stay unnormalized -- the
       conv ratio xf = conv(P*m)/conv(m) is scale invariant).
  2. coords: interp(448->224) + reflect-pad + 447-tap conv + P basis all
     folded host-side into two [448, 224] matrices; on device 16 small
     fp32 matmuls map raw marginals -> per-partition grid coordinates.
  3. separable bilinear grid-sample as two all-f32r matmul stages
     (moving N=256 keeps the f32r full-rate path; weights self-load so
     the PE sequencer stays out of the way). Tent weight matrices are
     built directly in [y, i] layout: coords replicated across
     partitions by a stride-0 DRAM read (pad lanes carry 1e9 so pad
     weights are exactly 0), u = |c-y| on Act (Abs with per-partition
     chunk-base bias), then min(u-1, 0) on DVE = NEGATED tents -- the
     sign cancels between the two matmul stages. No PE transposes.

Sharding: pure data-parallel, batch 64 -> 8 cores x 8.

DMA is the roofline (~87us of f32 traffic at 360 GB/s aggregate); DVE /
Act / Pool are balanced at ~46-50us each underneath it, and mm2 of
tile n issues after mm1 of tile n+1 so the PE never waits on the
PSUM->SBUF copies.
"""
import os
import sys

sys.path.insert(0, "/opt/trn_rl_repo")

import numpy as np
from contextlib import ExitStack

import concourse.bass as bass
import concourse.bacc as bacc
import concourse.tile as tile
from concourse import mybir, masks, bass_isa
from concourse.bass_utils import run_bass_kernel_spmd

F32 = mybir.dt.float32
F32R = mybir.dt.float32r
ALU = mybir.AluOpType
ACTF = mybir.ActivationFunctionType

SAM = 224
IN = 448
PAD = 223
GLOB = 670
KSIZE = 447
NCORES = 8
BSH = 8  # batch shard per core

_CACHE = {}

# expose the last run's results for test.py profiling
last_results = None


def _build_program():
    nc = bacc.Bacc("TRN2", num_devices=NCORES)

    data_in = nc.dram_tensor("data", (BSH, 3, IN, IN), F32R, kind="ExternalInput")
    att_in = nc.dram_tensor("att", (BSH, IN, IN), F32, kind="ExternalInput")
    wcm_in = nc.dram_tensor("wcm", (IN, 2, SAM), F32, kind="ExternalInput")
    nbcyc_in = nc.dram_tensor("nbcyc", (112, 4), F32, kind="ExternalInput")

    out_dram = nc.dram_tensor("out", (BSH, 3, SAM, SAM), F32, kind="ExternalOutput")
    pcd = nc.dram_tensor("pcd", (BSH, 4, 112), F32, kind="Internal")

    with tile.TileContext(nc) as tc, ExitStack() as ctx:
        consts = ctx.enter_context(tc.tile_pool(name="consts", bufs=1))
        apool = ctx.enter_context(tc.tile_pool(name="apool", bufs=5))
        dpool = ctx.enter_context(tc.tile_pool(name="dpool", bufs=10))
        fpool = ctx.enter_context(tc.tile_pool(name="fpool", bufs=2))
        arpool = ctx.enter_context(tc.tile_pool(name="arpool", bufs=2))
        mpool = ctx.enter_context(tc.tile_pool(name="mpool", bufs=1))
        ypool = ctx.enter_context(tc.tile_pool(name="ypool", bufs=4))
        wpool = ctx.enter_context(tc.tile_pool(name="wpool", bufs=2))
        btpool = ctx.enter_context(tc.tile_pool(name="btpool", bufs=4))
        opool = ctx.enter_context(tc.tile_pool(name="opool", bufs=3))
        ps1 = ctx.enter_context(tc.tile_pool(name="ps1", bufs=2, space="PSUM"))
        psA = ctx.enter_context(tc.tile_pool(name="psA", bufs=4, space="PSUM"))
        psB = ctx.enter_context(tc.tile_pool(name="psB", bufs=2, space="PSUM"))

        # small consts first on the scalar queue: dispatched at t=0,
        # transfers trivial, so the att loads own the DMA engines
        nbcyc = consts.tile([112, 4], F32)
        nc.scalar.dma_start(out=nbcyc, in_=nbcyc_in[:, :])
        ident = consts.tile([128, 128], F32)
        masks.make_identity(nc, ident[:])

        # ---------------- phase 1: marginals for all batches ----------------
        # marg64[p, cc, b] = y-profile value at y = cc*112+p  (max over x)
        # marg16x row b    = x-profile of batch b (replicated via all-reduce)
        marg64 = mpool.tile([112, 4, BSH], F32)
        margT = mpool.tile([112, 4, 16], F32)
        for b in range(BSH):
            att_t = apool.tile([112, 4, IN], F32, tag="att_t")
            nc.sync.dma_start(
                out=att_t, in_=att_in[b].rearrange("(cc p) x -> p cc x", p=112))
            # y-profile: max over x (free dim) on DVE
            nc.vector.tensor_reduce(
                out=marg64[:, :, b], in_=att_t, axis=mybir.AxisListType.X,
                op=ALU.max)
            # x-profile (max over all 448 y): the real Pool engine has no
            # elementwise ops, only ucode (all-reduce / affine_select), so
            # the cc-fold runs either as DVE maxes (path A, short chain:
            # used for the last batches on the critical tail) or entirely
            # off-DVE as a double all-reduce (path B): AR#1 replicates the
            # per-(cc,x) column max to every partition, mod-32-aligned Act
            # copies stack the four cc slices into four 32-partition
            # bands, AR#2 folds them.
            if b >= 5:
                f1 = fpool.tile([128, IN], F32, tag="f1")
                nc.vector.tensor_tensor(
                    out=f1[0:112, :], in0=att_t[:, 0, :], in1=att_t[:, 1, :],
                    op=ALU.max)
                f2 = fpool.tile([112, IN], F32, tag="f2")
                nc.vector.tensor_tensor(
                    out=f2, in0=att_t[:, 2, :], in1=att_t[:, 3, :], op=ALU.max)
                nc.vector.tensor_tensor(
                    out=f1[0:112, :], in0=f1[0:112, :], in1=f2, op=ALU.max)
            else:
                ar1 = fpool.tile([112, 4, IN], F32, tag="ar1")
                nc.gpsimd.partition_all_reduce(
                    out_ap=ar1, in_ap=att_t, channels=112,
                    reduce_op=bass_isa.ReduceOp.max)
                # ar1 is replicated: partitions 0:32 hold every cc slice
                f1 = fpool.tile([128, IN], F32, tag="f1")
                for cc in range(4):
                    nc.scalar.copy(
                        out=f1[32 * cc:32 * cc + 32, :],
                        in_=ar1[0:32, cc, :])
            arx = arpool.tile([112, IN], F32, tag="arx")
            nc.gpsimd.partition_all_reduce(
                out_ap=arx, in_ap=f1[0:112, :], channels=112,
                reduce_op=bass_isa.ReduceOp.max)
            # transpose the replicated profile so x lands on partitions,
            # then one strided copy of column 0 fills margT[:, :, b]
            atr = ps1.tile([112, 4, 112], F32, tag="p1ps")
            for xc in range(4):
                nc.tensor.transpose(
                    atr[:, xc, :], arx[:, xc * 112:(xc + 1) * 112],
                    ident[0:112, 0:112])
            nc.scalar.copy(out=margT[:, :, b:b + 1], in_=atr[:, :, 0:1])

        # the big fused-conv const rides the sync queue between att and data
        wcm = consts.tile([112, 4, 2, SAM], F32)
        nc.sync.dma_start(
            out=wcm, in_=wcm_in.rearrange("(xc p) s o -> p xc s o", p=112))

        # ---------------- coords: one fused linear map + ratio ----------
        # margT[p, xc, r]: marginal value at x = xc*112+p for row r
        # (r 0..7: x-profiles -> j coords; r 8..15: y-profiles -> i coords)
        nc.vector.tensor_copy(out=margT[:, :, 8:16], in_=marg64)

        # px_ps[p, oh, 0, r] = conv(m)[oh*112+p], [.., 1, r] = conv(P*m)
        px_ps = ps1.tile([112, 2, 2, 16], F32, tag="p1ps")
        for s in range(2):
            for oh in range(2):
                for xc in range(4):
                    nc.tensor.matmul(
                        px_ps[:, oh, s, :],
                        lhsT=wcm[:, xc, s, oh * 112:(oh + 1) * 112],
                        rhs=margT[:, xc, :],
                        start=(xc == 0), stop=(xc == 3))
        pxs = mpool.tile([112, 2, 2, 16], F32)
        nc.vector.tensor_copy(out=pxs, in_=px_ps)
        rec = mpool.tile([112, 2, 16], F32)
        nc.vector.reciprocal(out=rec, in_=pxs[:, :, 0, :])
        pc = mpool.tile([112, 2, 16], F32)
        nc.vector.scalar_tensor_tensor(
            out=pc, in0=rec, scalar=447.0, in1=pxs[:, :, 1, :],
            op0=ALU.mult, op1=ALU.mult)
        nc.vector.tensor_scalar(
            out=pc, in0=pc, scalar1=0.0, scalar2=447.0,
            op0=ALU.max, op1=ALU.min)

        # coords to rows: pcT[ih*16 + r, p] = pc[p, ih, r]. The rows are
        # staged to DRAM reordered by batch (pcd[b] = [i_lo, i_hi, j_lo,
        # j_hi]); each batch then broadcasts its 448 coords across all
        # partitions with one stride-0 DRAM read (the proven mechanism --
        # engine ops cannot shift partitions off 32-multiples, and the
        # gpsimd ucode broadcast misbehaves in-context).
        pcT_ps = ps1.tile([32, 112], F32, tag="p1ps")
        nc.tensor.transpose(pcT_ps, pc, ident[0:112, 0:112])
        pcT = mpool.tile([32, 112], F32)
        nc.vector.tensor_copy(out=pcT, in_=pcT_ps)
        for g, q in ((0, 2), (1, 0), (2, 3), (3, 1)):
            # rows 8g..8g+7 hold (x if g in {0,2} else y) coords, half g//2
            nc.sync.dma_start(
                out=bass.AP(pcd, q * 112, [[448, 8], [1, 112]]),
                in_=pcT[8 * g:8 * (g + 1), :])


        # ---------------- phase B: grid-sample ----------------
        wargs = {}

        def build_weights(b):
            # tent weights in [y, i] layout: w[p, yc, i] =
            # relu(1 - |c_i - (112*yc + p)|), c replicated across
            # partitions by gpsimd partition_broadcast. axis 0: wy
            # (coords row 8+b), axis 1: wx (row b). f32r, zero pads
            # to N=256 for the full-rate f32r moving path.
            # one stride-0 DRAM read replicates this batch's 448 coords
            # (i then j) onto every partition; pad lanes carry 1e9 so the
            # tents (and the f32r pad columns) are exactly zero there
            ycb = ypool.tile([112, 2, 256], F32, tag="ycb")
            nc.vector.memset(ycb[:, :, 224:256], 1e9)
            nc.sync.dma_start(
                out=ycb[:, :, 0:224],
                in_=bass.AP(pcd, b * 448, [[0, 112], [224, 2], [1, 224]]))
            # wsb pad columns [224:256) are never written: the matmuls
            # stream them into PSUM columns that no copy ever reads, so
            # their values are irrelevant (and f32r memset is ISA-illegal)
            wsb0 = wpool.tile([112, 4, 256], F32R, tag="w0")
            wsb1 = wpool.tile([112, 4, 256], F32R, tag="w1")
            # abs_max is not a valid hw TensorScalar op, and f32r memset
            # is illegal, so the tents are built NEGATED: u = |c - y| via
            # the Act Abs function, then min(u - 1, 0) = -relu(1 - u).
            # mm1 and mm2 both use negated weights; the signs cancel.
            for yc in range(4):
                u = ypool.tile([112, 2, 256], F32, tag="u")
                nc.scalar.activation(
                    out=u, in_=ycb, func=ACTF.Abs,
                    bias=nbcyc[:, yc:yc + 1], scale=1.0)
                nc.vector.tensor_scalar(
                    out=wsb0[:, yc, :], in0=u[:, 0, :], scalar1=1.0,
                    scalar2=0.0, op0=ALU.subtract, op1=ALU.min)
                nc.vector.tensor_scalar(
                    out=wsb1[:, yc, :], in0=u[:, 1, :], scalar1=1.0,
                    scalar2=0.0, op0=ALU.subtract, op1=ALU.min)
            wargs[(b, 0)] = wsb0
            wargs[(b, 1)] = wsb1

        def stage2(bt, wx, b, c):
            # x-axis sampling of the y-interpolated tile + output staging
            ob = psB.tile([112, 2, 256], F32, tag="ob")
            for ih in range(2):
                for xc in range(4):
                    nc.tensor.matmul(
                        ob[:, ih, :],
                        lhsT=bt[:, xc, ih * 112:(ih + 1) * 112],
                        rhs=wx[:, xc, :],
                        start=(xc == 0), stop=(xc == 3))
            osb = opool.tile([112, 2, SAM], F32, tag="osb")
            if c == 0:
                nc.vector.tensor_copy(out=osb, in_=ob[:, :, 0:224])
            else:
                nc.scalar.copy(out=osb, in_=ob[:, :, 0:224])
            nc.scalar.dma_start(
                out=out_dram[b, c].rearrange("(ih p) j -> p ih j", p=112),
                in_=osb)

        # software pipeline: weights one batch ahead; mm2 of tile n issues
        # after mm1 of tile n+1 so the PE never waits on the bt copies
        build_weights(0)
        pending = None
        for b in range(BSH):
            wy = wargs.pop((b, 0))
            wx = wargs.pop((b, 1))
            for c in range(3):
                if c == 0 and b + 1 < BSH:
                    build_weights(b + 1)
                at = dpool.tile([112, 4, IN], F32R, tag="at")
                nc.sync.dma_start(
                    out=at, in_=data_in[b, c].rearrange("(cc p) x -> p cc x", p=112))

                bt = btpool.tile([112, 4, SAM], F32R, tag="bt")
                for xc in range(4):
                    btp = psA.tile([112, 256], F32, tag="btp")
                    for yc in range(4):
                        nc.tensor.matmul(
                            btp,
                            lhsT=at[:, yc, xc * 112:(xc + 1) * 112],
                            rhs=wy[:, yc, :],
                            start=(yc == 0), stop=(yc == 3))
                    if xc % 2 == 0:
                        nc.scalar.copy(out=bt[:, xc, :], in_=btp[:, 0:224])
                    else:
                        nc.vector.tensor_copy(out=bt[:, xc, :], in_=btp[:, 0:224])
                if pending is not None:
                    stage2(*pending)
                pending = (bt, wx, b, c)
        stage2(*pending)
    nc.compile()
    return nc


def _static_consts(filter_w: np.ndarray):
    # fuse interp(448->224) + reflect-pad(->670) + 447-tap conv + P basis
    # into two [448, 224] matrices:  px = M1^T m,  pxP = MP^T m
    fw = filter_w.astype(np.float64)
    L = np.zeros((SAM, IN), dtype=np.float64)          # msn = L m
    j = np.arange(SAM)
    w = j / float(PAD)
    L[j, 2 * j] = 1.0 - w
    L[j, np.minimum(2 * j + 1, IN - 1)] += w
    S = np.zeros((GLOB, SAM), dtype=np.float64)        # sig = S msn
    S[np.arange(PAD), PAD - np.arange(PAD)] = 1.0      # left reflect
    S[PAD + np.arange(SAM), np.arange(SAM)] = 1.0      # center
    S[KSIZE + np.arange(PAD), PAD - 1 - np.arange(PAD)] = 1.0  # right reflect
    wm = np.zeros((GLOB, SAM), dtype=np.float64)       # conv: px = wm^T sig
    g = np.arange(GLOB)[:, None]
    o = np.arange(SAM)[None, :]
    k = g - o
    valid = (k >= 0) & (k < KSIZE)
    wm[valid] = fw[k[valid]]
    P = (np.arange(GLOB, dtype=np.float64) - PAD) / float(PAD)
    SL = S @ L                                         # [670, 448]
    M1 = SL.T @ wm                                     # [448, 224]
    MP = (P[:, None] * SL).T @ wm
    wcm = np.stack([M1, MP], axis=1).astype(np.float32)  # [448, 2, 224]
    nbcyc = -(np.arange(112, dtype=np.float32)[:, None]
              + 112.0 * np.arange(4, dtype=np.float32)[None, :])
    return {"wcm": wcm, "nbcyc": nbcyc}


def kernel(data: np.ndarray, structure_att: np.ndarray,
           filter_w: np.ndarray) -> np.ndarray:
    global last_results
    data = np.ascontiguousarray(data, dtype=np.float32)
    structure_att = np.ascontiguousarray(structure_att, dtype=np.float32)
    filter_w = np.ascontiguousarray(filter_w, dtype=np.float32)

    if "nc" not in _CACHE:
        _CACHE["nc"] = _build_program()
    nc = _CACHE["nc"]

    consts = _static_consts(filter_w)
    in_maps = []
    for core in range(NCORES):
        sl = slice(core * BSH, (core + 1) * BSH)
        in_maps.append({
            "data": data[sl], "att": structure_att[sl], **consts,
        })

    res = run_bass_kernel_spmd(nc, in_maps, core_ids=list(range(NCORES)))
    last_results = res
    out = np.concatenate([res.results[i]["out"] for i in range(NCORES)], axis=0)
    return out


# revision 4
# speedup vs baseline: 1.0142x; 1.0142x over previous
"""Trainium2 Bass kernel for nn_MASNET2 (structure-attention warped resampling).

Per batch:
  1. axis-max marginals of structure_att:
       y-profile: DVE free-dim max; x-profile: cc-fold (DVE+Pool) +
       gpsimd partition_all_reduce (profiles stay unnormalized -- the
       conv ratio xf = conv(P*m)/conv(m) is scale invariant).
  2. coords: interp(448->224) + reflect-pad + 447-tap conv + P basis all
     folded host-side into two [448, 224] matrices; on device 16 small
     fp32 matmuls map raw marginals -> per-partition grid coordinates.
  3. separable bilinear grid-sample as two all-f32r matmul stages
     (moving N=256 keeps the f32r full-rate path; weights self-load so
     the PE sequencer stays out of the way). Tent weight matrices are
     built directly in [y, i] layout: coords replicated across
     partitions with gpsimd partition_broadcast, |c-y| on DVE with a
     per-partition chunk-base bias, relu(1-u) on Act straight into the
     weight tile. No PE transposes, no PSUM staging.

Sharding: pure data-parallel, batch 64 -> 8 cores x 8.

DMA is the roofline (~87us of f32 traffic at 360 GB/s aggregate); DVE /
Act / Pool are balanced at ~46-50us each underneath it, and mm2 of
tile n issues after mm1 of tile n+1 so the PE never waits on the
PSUM->SBUF copies.
"""
import os
import sys

sys.path.insert(0, "/opt/trn_rl_repo")

import numpy as np
from contextlib import ExitStack

import concourse.bass as bass
import concourse.bacc as bacc
import concourse.tile as tile
from concourse import mybir, masks, bass_isa
from concourse.bass_utils import run_bass_kernel_spmd

F32 = mybir.dt.float32
F32R = mybir.dt.float32r
ALU = mybir.AluOpType
ACTF = mybir.ActivationFunctionType

SAM = 224
IN = 448
PAD = 223
GLOB = 670
KSIZE = 447
NCORES = 8
BSH = 8  # batch shard per core

_CACHE = {}

# expose the last run's results for test.py profiling
last_results = None


def _build_program():
    nc = bacc.Bacc("TRN2", num_devices=NCORES)

    data_in = nc.dram_tensor("data", (BSH, 3, IN, IN), F32R, kind="ExternalInput")
    att_in = nc.dram_tensor("att", (BSH, IN, IN), F32, kind="ExternalInput")
    wcm_in = nc.dram_tensor("wcm", (IN, 2, SAM), F32, kind="ExternalInput")
    nbcyc_in = nc.dram_tensor("nbcyc", (112, 4), F32, kind="ExternalInput")

    out_dram = nc.dram_tensor("out", (BSH, 3, SAM, SAM), F32, kind="ExternalOutput")
    pcd = nc.dram_tensor("pcd", (BSH, 4, 112), F32, kind="Internal")

    with tile.TileContext(nc) as tc, ExitStack() as ctx:
        consts = ctx.enter_context(tc.tile_pool(name="consts", bufs=1))
        apool = ctx.enter_context(tc.tile_pool(name="apool", bufs=5))
        dpool = ctx.enter_context(tc.tile_pool(name="dpool", bufs=10))
        fpool = ctx.enter_context(tc.tile_pool(name="fpool", bufs=2))
        arpool = ctx.enter_context(tc.tile_pool(name="arpool", bufs=2))
        mpool = ctx.enter_context(tc.tile_pool(name="mpool", bufs=1))
        ypool = ctx.enter_context(tc.tile_pool(name="ypool", bufs=4))
        wpool = ctx.enter_context(tc.tile_pool(name="wpool", bufs=2))
        btpool = ctx.enter_context(tc.tile_pool(name="btpool", bufs=4))
        opool = ctx.enter_context(tc.tile_pool(name="opool", bufs=3))
        ps1 = ctx.enter_context(tc.tile_pool(name="ps1", bufs=2, space="PSUM"))
        psA = ctx.enter_context(tc.tile_pool(name="psA", bufs=4, space="PSUM"))
        psB = ctx.enter_context(tc.tile_pool(name="psB", bufs=2, space="PSUM"))

        # small consts first on the scalar queue: dispatched at t=0,
        # transfers trivial, so the att loads own the DMA engines
        nbcyc = consts.tile([112, 4], F32)
        nc.scalar.dma_start(out=nbcyc, in_=nbcyc_in[:, :])
        ident = consts.tile([128, 128], F32)
        masks.make_identity(nc, ident[:])

        # ---------------- phase 1: marginals for all batches ----------------
        # marg64[p, cc, b] = y-profile value at y = cc*112+p  (max over x)
        # marg16x row b    = x-profile of batch b (replicated via all-reduce)
        marg64 = mpool.tile([112, 4, BSH], F32)
        margT = mpool.tile([112, 4, 16], F32)
        for b in range(BSH):
            att_t = apool.tile([112, 4, IN], F32, tag="att_t")
            nc.sync.dma_start(
                out=att_t, in_=att_in[b].rearrange("(cc p) x -> p cc x", p=112))
            # y-profile: max over x (free dim) on DVE
            nc.vector.tensor_reduce(
                out=marg64[:, :, b], in_=att_t, axis=mybir.AxisListType.X,
                op=ALU.max)
            # x-profile (max over all 448 y): the real Pool engine has no
            # elementwise ops, only ucode (all-reduce / affine_select), so
            # the cc-fold runs either as DVE maxes (path A, short chain:
            # used for the last batches on the critical tail) or entirely
            # off-DVE as a double all-reduce (path B): AR#1 replicates the
            # per-(cc,x) column max to every partition, mod-32-aligned Act
            # copies stack the four cc slices into four 32-partition
            # bands, AR#2 folds them.
            if b >= 5:
                f1 = fpool.tile([128, IN], F32, tag="f1")
                nc.vector.tensor_tensor(
                    out=f1[0:112, :], in0=att_t[:, 0, :], in1=att_t[:, 1, :],
                    op=ALU.max)
                f2 = fpool.tile([112, IN], F32, tag="f2")
                nc.vector.tensor_tensor(
                    out=f2, in0=att_t[:, 2, :], in1=att_t[:, 3, :], op=ALU.max)
                nc.vector.tensor_tensor(
                    out=f1[0:112, :], in0=f1[0:112, :], in1=f2, op=ALU.max)
            else:
                ar1 = fpool.tile([112, 4, IN], F32, tag="ar1")
                nc.gpsimd.partition_all_reduce(
                    out_ap=ar1, in_ap=att_t, channels=112,
                    reduce_op=bass_isa.ReduceOp.max)
                # ar1 is replicated: partitions 0:32 hold every cc slice
                f1 = fpool.tile([128, IN], F32, tag="f1")
                for cc in range(4):
                    nc.scalar.copy(
                        out=f1[32 * cc:32 * cc + 32, :],
                        in_=ar1[0:32, cc, :])
            arx = arpool.tile([112, IN], F32, tag="arx")
            nc.gpsimd.partition_all_reduce(
                out_ap=arx, in_ap=f1[0:112, :], channels=112,
                reduce_op=bass_isa.ReduceOp.max)
            # transpose the replicated profile so x lands on partitions,
            # then one strided copy of column 0 fills margT[:, :, b]
            atr = ps1.tile([112, 4, 112], F32, tag="p1ps")
            for xc in range(4):
                nc.tensor.transpose(
                    atr[:, xc, :], arx[:, xc * 112:(xc + 1) * 112],
                    ident[0:112, 0:112])
            nc.scalar.copy(out=margT[:, :, b:b + 1], in_=atr[:, :, 0:1])

        # the big fused-conv const rides the sync queue between att and data
        wcm = consts.tile([112, 4, 2, SAM], F32)
        nc.sync.dma_start(
            out=wcm, in_=wcm_in.rearrange("(xc p) s o -> p xc s o", p=112))

        # ---------------- coords: one fused linear map + ratio ----------
        # margT[p, xc, r]: marginal value at x = xc*112+p for row r
        # (r 0..7: x-profiles -> j coords; r 8..15: y-profiles -> i coords)
        nc.vector.tensor_copy(out=margT[:, :, 8:16], in_=marg64)

        # px_ps[p, oh, 0, r] = conv(m)[oh*112+p], [.., 1, r] = conv(P*m)
        px_ps = ps1.tile([112, 2, 2, 16], F32, tag="p1ps")
        for s in range(2):
            for oh in range(2):
                for xc in range(4):
                    nc.tensor.matmul(
                        px_ps[:, oh, s, :],
                        lhsT=wcm[:, xc, s, oh * 112:(oh + 1) * 112],
                        rhs=margT[:, xc, :],
                        start=(xc == 0), stop=(xc == 3))
        pxs = mpool.tile([112, 2, 2, 16], F32)
        nc.scalar.copy(out=pxs, in_=px_ps)
        rec = mpool.tile([112, 2, 16], F32)
        nc.vector.reciprocal(out=rec, in_=pxs[:, :, 0, :])
        pc = mpool.tile([112, 2, 16], F32)
        nc.vector.scalar_tensor_tensor(
            out=pc, in0=rec, scalar=447.0, in1=pxs[:, :, 1, :],
            op0=ALU.mult, op1=ALU.mult)
        # clip and reorder to batch-major rows in one op: pc2[p, b, q] with
        # q = axis*2 + ih (r 8+b -> axis 0, r b -> axis 1)
        pc2 = mpool.tile([112, 8, 4], F32)
        pc2_view = bass.AP(pc2.tensor, pc2.offset,
                           [list(pc2.ap[0]), [1, 2], [2, 2], [4, 8]])
        pc_view = bass.AP(pc.tensor, pc.offset,
                          [list(pc.ap[0]), [16, 2], [-8, 2], [1, 8]])
        nc.vector.tensor_scalar(
            out=pc2_view, in0=bass.AP(pc.tensor, pc.offset + 8,
                                      [list(pc.ap[0]), [16, 2], [-8, 2], [1, 8]]),
            scalar1=0.0, scalar2=447.0, op0=ALU.max, op1=ALU.min)

        # coords to rows, already batch-major: pcT2[b*4 + q, p]; staged to
        # DRAM with a single contiguous DMA, then each batch broadcasts its
        # 448 coords with one stride-0 read
        pcT_ps = ps1.tile([32, 112], F32, tag="p1ps")
        nc.tensor.transpose(pcT_ps, pc2, ident[0:112, 0:112])
        pcT = mpool.tile([32, 112], F32)
        nc.scalar.copy(out=pcT, in_=pcT_ps)
        nc.sync.dma_start(
            out=bass.AP(pcd, 0, [[112, 32], [1, 112]]), in_=pcT)


        # ---------------- phase B: grid-sample ----------------
        wargs = {}

        def build_weights(b):
            # tent weights in [y, i] layout: w[p, yc, i] =
            # relu(1 - |c_i - (112*yc + p)|), c replicated across
            # partitions by gpsimd partition_broadcast. axis 0: wy
            # (coords row 8+b), axis 1: wx (row b). f32r, zero pads
            # to N=256 for the full-rate f32r moving path.
            # one stride-0 DRAM read replicates this batch's 448 coords
            # (i then j) onto every partition; pad lanes carry 1e9 so the
            # tents (and the f32r pad columns) are exactly zero there
            ycb = ypool.tile([112, 2, 256], F32, tag="ycb")
            nc.vector.memset(ycb[:, :, 224:256], 1e9)
            nc.sync.dma_start(
                out=ycb[:, :, 0:224],
                in_=bass.AP(pcd, b * 448, [[0, 112], [224, 2], [1, 224]]))
            # wsb pad columns [224:256) are never written: the matmuls
            # stream them into PSUM columns that no copy ever reads, so
            # their values are irrelevant (and f32r memset is ISA-illegal)
            wsb0 = wpool.tile([112, 4, 256], F32R, tag="w0")
            wsb1 = wpool.tile([112, 4, 256], F32R, tag="w1")
            # abs_max is not a valid hw TensorScalar op, and f32r memset
            # is illegal, so the tents are built NEGATED: u = |c - y| via
            # the Act Abs function, then min(u - 1, 0) = -relu(1 - u).
            # mm1 and mm2 both use negated weights; the signs cancel.
            for yc in range(4):
                u = ypool.tile([112, 2, 256], F32, tag="u")
                nc.scalar.activation(
                    out=u, in_=ycb, func=ACTF.Abs,
                    bias=nbcyc[:, yc:yc + 1], scale=1.0)
                nc.vector.tensor_scalar(
                    out=wsb0[:, yc, :], in0=u[:, 0, :], scalar1=1.0,
                    scalar2=0.0, op0=ALU.subtract, op1=ALU.min)
                nc.vector.tensor_scalar(
                    out=wsb1[:, yc, :], in0=u[:, 1, :], scalar1=1.0,
                    scalar2=0.0, op0=ALU.subtract, op1=ALU.min)
            wargs[(b, 0)] = wsb0
            wargs[(b, 1)] = wsb1

        def stage2(bt, wx, b, c):
            # x-axis sampling of the y-interpolated tile + output staging
            ob = psB.tile([112, 2, 256], F32, tag="ob")
            for ih in range(2):
                for xc in range(4):
                    nc.tensor.matmul(
                        ob[:, ih, :],
                        lhsT=bt[:, xc, ih * 112:(ih + 1) * 112],
                        rhs=wx[:, xc, :],
                        start=(xc == 0), stop=(xc == 3))
            osb = opool.tile([112, 2, SAM], F32, tag="osb")
            if c == 0:
                nc.vector.tensor_copy(out=osb, in_=ob[:, :, 0:224])
            else:
                nc.scalar.copy(out=osb, in_=ob[:, :, 0:224])
            nc.scalar.dma_start(
                out=out_dram[b, c].rearrange("(ih p) j -> p ih j", p=112),
                in_=osb)

        # software pipeline: weights one batch ahead; mm2 of tile n issues
        # after mm1 of tile n+1 so the PE never waits on the bt copies
        build_weights(0)
        pending = None
        for b in range(BSH):
            wy = wargs.pop((b, 0))
            wx = wargs.pop((b, 1))
            for c in range(3):
                if c == 0 and b + 1 < BSH:
                    build_weights(b + 1)
                at = dpool.tile([112, 4, IN], F32R, tag="at")
                nc.sync.dma_start(
                    out=at, in_=data_in[b, c].rearrange("(cc p) x -> p cc x", p=112))

                bt = btpool.tile([112, 4, SAM], F32R, tag="bt")
                for xc in range(4):
                    btp = psA.tile([112, 256], F32, tag="btp")
                    for yc in range(4):
                        nc.tensor.matmul(
                            btp,
                            lhsT=at[:, yc, xc * 112:(xc + 1) * 112],
                            rhs=wy[:, yc, :],
                            start=(yc == 0), stop=(yc == 3))
                    if xc % 2 == 0:
                        nc.scalar.copy(out=bt[:, xc, :], in_=btp[:, 0:224])
                    else:
                        nc.vector.tensor_copy(out=bt[:, xc, :], in_=btp[:, 0:224])
                if pending is not None:
                    stage2(*pending)
                pending = (bt, wx, b, c)
        stage2(*pending)
    nc.compile()
    return nc


def _static_consts(filter_w: np.ndarray):
    # fuse interp(448->224) + reflect-pad(->670) + 447-tap conv + P basis
    # into two [448, 224] matrices:  px = M1^T m,  pxP = MP^T m
    fw = filter_w.astype(np.float64)
    L = np.zeros((SAM, IN), dtype=np.float64)          # msn = L m
    j = np.arange(SAM)
    w = j / float(PAD)
    L[j, 2 * j] = 1.0 - w
    L[j, np.minimum(2 * j + 1, IN - 1)] += w
    S = np.zeros((GLOB, SAM), dtype=np.float64)        # sig = S msn
    S[np.arange(PAD), PAD - np.arange(PAD)] = 1.0      # left reflect
    S[PAD + np.arange(SAM), np.arange(SAM)] = 1.0      # center
    S[KSIZE + np.arange(PAD), PAD - 1 - np.arange(PAD)] = 1.0  # right reflect
    wm = np.zeros((GLOB, SAM), dtype=np.float64)       # conv: px = wm^T sig
    g = np.arange(GLOB)[:, None]
    o = np.arange(SAM)[None, :]
    k = g - o
    valid = (k >= 0) & (k < KSIZE)
    wm[valid] = fw[k[valid]]
    P = (np.arange(GLOB, dtype=np.float64) - PAD) / float(PAD)
    SL = S @ L                                         # [670, 448]
    M1 = SL.T @ wm                                     # [448, 224]
    MP = (P[:, None] * SL).T @ wm
    wcm = np.stack([M1, MP], axis=1).astype(np.float32)  # [448, 2, 224]
    nbcyc = -(np.arange(112, dtype=np.float32)[:, None]
              + 112.0 * np.arange(4, dtype=np.float32)[None, :])
    return {"wcm": wcm, "nbcyc": nbcyc}


def kernel(data: np.ndarray, structure_att: np.ndarray,
           filter_w: np.ndarray) -> np.ndarray:
    global last_results
    data = np.ascontiguousarray(data, dtype=np.float32)
    structure_att = np.ascontiguousarray(structure_att, dtype=np.float32)
    filter_w = np.ascontiguousarray(filter_w, dtype=np.float32)

    if "nc" not in _CACHE:
        _CACHE["nc"] = _build_program()
    nc = _CACHE["nc"]

    consts = _static_consts(filter_w)
    in_maps = []
    for core in range(NCORES):
        sl = slice(core * BSH, (core + 1) * BSH)
        in_maps.append({
            "data": data[sl], "att": structure_att[sl], **consts,
        })

    res = run_bass_kernel_spmd(nc, in_maps, core_ids=list(range(NCORES)))
    last_results = res
    out = np.concatenate([res.results[i]["out"] for i in range(NCORES)], axis=0)
    return out


# revision 5
# speedup vs baseline: 1.0408x; 1.0263x over previous
"""Trainium2 Bass kernel for nn_MASNET2 (structure-attention warped resampling).

Per batch:
  1. axis-max marginals of structure_att:
       y-profile: DVE free-dim max; x-profile: cc-fold (DVE+Pool) +
       gpsimd partition_all_reduce (profiles stay unnormalized -- the
       conv ratio xf = conv(P*m)/conv(m) is scale invariant).
  2. coords: interp(448->224) + reflect-pad + 447-tap conv + P basis all
     folded host-side into two [448, 224] matrices; on device 16 small
     fp32 matmuls map raw marginals -> per-partition grid coordinates.
  3. separable bilinear grid-sample as two all-f32r matmul stages
     (moving N=256 keeps the f32r full-rate path; weights self-load so
     the PE sequencer stays out of the way). Tent weight matrices are
     built directly in [y, i] layout: coords replicated across
     partitions with gpsimd partition_broadcast, |c-y| on DVE with a
     per-partition chunk-base bias, relu(1-u) on Act straight into the
     weight tile. No PE transposes, no PSUM staging.

Sharding: pure data-parallel, batch 64 -> 8 cores x 8.

DMA is the roofline (~87us of f32 traffic at 360 GB/s aggregate); DVE /
Act / Pool are balanced at ~46-50us each underneath it, and mm2 of
tile n issues after mm1 of tile n+1 so the PE never waits on the
PSUM->SBUF copies.
"""
import os
import sys

sys.path.insert(0, "/opt/trn_rl_repo")

import numpy as np
from contextlib import ExitStack

import concourse.bass as bass
import concourse.bacc as bacc
import concourse.tile as tile
from concourse import mybir, masks, bass_isa
from concourse.bass_utils import run_bass_kernel_spmd

F32 = mybir.dt.float32
F32R = mybir.dt.float32r
ALU = mybir.AluOpType
ACTF = mybir.ActivationFunctionType

SAM = 224
IN = 448
PAD = 223
GLOB = 670
KSIZE = 447
NCORES = 8
BSH = 8  # batch shard per core

_CACHE = {}

# expose the last run's results for test.py profiling
last_results = None


def _build_program():
    nc = bacc.Bacc("TRN2", num_devices=NCORES)

    data_in = nc.dram_tensor("data", (BSH, 3, IN, IN), F32R, kind="ExternalInput")
    att_in = nc.dram_tensor("att", (BSH, IN, IN), F32, kind="ExternalInput")
    wcm_in = nc.dram_tensor("wcm", (IN, 2, SAM), F32, kind="ExternalInput")
    nbcyc_in = nc.dram_tensor("nbcyc", (112, 4), F32, kind="ExternalInput")

    out_dram = nc.dram_tensor("out", (BSH, 3, SAM, SAM), F32, kind="ExternalOutput")
    pcd = nc.dram_tensor("pcd", (BSH, 4, 112), F32, kind="Internal")

    with tile.TileContext(nc) as tc, ExitStack() as ctx:
        consts = ctx.enter_context(tc.tile_pool(name="consts", bufs=1))
        apool = ctx.enter_context(tc.tile_pool(name="apool", bufs=5))
        dpool = ctx.enter_context(tc.tile_pool(name="dpool", bufs=10))
        fpool = ctx.enter_context(tc.tile_pool(name="fpool", bufs=2))
        arpool = ctx.enter_context(tc.tile_pool(name="arpool", bufs=2))
        mpool = ctx.enter_context(tc.tile_pool(name="mpool", bufs=1))
        ypool = ctx.enter_context(tc.tile_pool(name="ypool", bufs=4))
        wpool = ctx.enter_context(tc.tile_pool(name="wpool", bufs=2))
        btpool = ctx.enter_context(tc.tile_pool(name="btpool", bufs=4))
        opool = ctx.enter_context(tc.tile_pool(name="opool", bufs=3))
        ps1 = ctx.enter_context(tc.tile_pool(name="ps1", bufs=2, space="PSUM"))
        psA = ctx.enter_context(tc.tile_pool(name="psA", bufs=4, space="PSUM"))
        psB = ctx.enter_context(tc.tile_pool(name="psB", bufs=2, space="PSUM"))

        # small consts first on the scalar queue: dispatched at t=0,
        # transfers trivial, so the att loads own the DMA engines
        nbcyc = consts.tile([112, 4], F32)
        nc.scalar.dma_start(out=nbcyc, in_=nbcyc_in[:, :])
        ident = consts.tile([128, 128], F32)
        masks.make_identity(nc, ident[:])

        # ---------------- phase 1: marginals for all batches ----------------
        # marg64[p, cc, b] = y-profile value at y = cc*112+p  (max over x)
        # marg16x row b    = x-profile of batch b (replicated via all-reduce)
        marg64 = mpool.tile([112, 4, BSH], F32)
        margT = mpool.tile([112, 4, 16], F32)
        for b in range(BSH):
            att_t = apool.tile([112, 4, IN], F32, tag="att_t")
            nc.sync.dma_start(
                out=att_t, in_=att_in[b].rearrange("(cc p) x -> p cc x", p=112))
            # y-profile: max over x (free dim) on DVE
            nc.vector.tensor_reduce(
                out=marg64[:, :, b], in_=att_t, axis=mybir.AxisListType.X,
                op=ALU.max)
            # x-profile (max over all 448 y): the real Pool engine has no
            # elementwise ops, only ucode (all-reduce / affine_select), so
            # the cc-fold runs either as DVE maxes (path A, short chain:
            # used for the last batches on the critical tail) or entirely
            # off-DVE as a double all-reduce (path B): AR#1 replicates the
            # per-(cc,x) column max to every partition, mod-32-aligned Act
            # copies stack the four cc slices into four 32-partition
            # bands, AR#2 folds them.
            if b >= 5:
                f1 = fpool.tile([128, IN], F32, tag="f1")
                nc.vector.tensor_tensor(
                    out=f1[0:112, :], in0=att_t[:, 0, :], in1=att_t[:, 1, :],
                    op=ALU.max)
                f2 = fpool.tile([112, IN], F32, tag="f2")
                nc.vector.tensor_tensor(
                    out=f2, in0=att_t[:, 2, :], in1=att_t[:, 3, :], op=ALU.max)
                nc.vector.tensor_tensor(
                    out=f1[0:112, :], in0=f1[0:112, :], in1=f2, op=ALU.max)
            else:
                ar1 = fpool.tile([112, 4, IN], F32, tag="ar1")
                nc.gpsimd.partition_all_reduce(
                    out_ap=ar1, in_ap=att_t, channels=112,
                    reduce_op=bass_isa.ReduceOp.max)
                # ar1 is replicated: partitions 0:32 hold every cc slice
                f1 = fpool.tile([128, IN], F32, tag="f1")
                for cc in range(4):
                    nc.scalar.copy(
                        out=f1[32 * cc:32 * cc + 32, :],
                        in_=ar1[0:32, cc, :])
            arx = arpool.tile([112, IN], F32, tag="arx")
            nc.gpsimd.partition_all_reduce(
                out_ap=arx, in_ap=f1[0:112, :], channels=112,
                reduce_op=bass_isa.ReduceOp.max)
            # transpose the replicated profile so x lands on partitions,
            # then one strided copy of column 0 fills margT[:, :, b]
            atr = ps1.tile([112, 4, 112], F32, tag="p1ps")
            for xc in range(4):
                nc.tensor.transpose(
                    atr[:, xc, :], arx[:, xc * 112:(xc + 1) * 112],
                    ident[0:112, 0:112])
            nc.scalar.copy(out=margT[:, :, b:b + 1], in_=atr[:, :, 0:1])

        # the big fused-conv const rides the sync queue between att and data
        wcm = consts.tile([112, 4, 2, SAM], F32)
        nc.sync.dma_start(
            out=wcm, in_=wcm_in.rearrange("(xc p) s o -> p xc s o", p=112))

        # ---------------- coords: one fused linear map + ratio ----------
        # margT[p, xc, r]: marginal value at x = xc*112+p for row r
        # (r 0..7: x-profiles -> j coords; r 8..15: y-profiles -> i coords)
        nc.vector.tensor_copy(out=margT[:, :, 8:16], in_=marg64)

        # px_ps[p, oh, 0, r] = conv(m)[oh*112+p], [.., 1, r] = conv(P*m)
        px_ps = ps1.tile([112, 2, 2, 16], F32, tag="p1ps")
        for s in range(2):
            for oh in range(2):
                for xc in range(4):
                    nc.tensor.matmul(
                        px_ps[:, oh, s, :],
                        lhsT=wcm[:, xc, s, oh * 112:(oh + 1) * 112],
                        rhs=margT[:, xc, :],
                        start=(xc == 0), stop=(xc == 3))
        pxs = mpool.tile([112, 2, 2, 16], F32)
        nc.scalar.copy(out=pxs, in_=px_ps)
        rec = mpool.tile([112, 2, 16], F32)
        nc.vector.reciprocal(out=rec, in_=pxs[:, :, 0, :])
        pc = mpool.tile([112, 2, 16], F32)
        nc.vector.scalar_tensor_tensor(
            out=pc, in0=rec, scalar=447.0, in1=pxs[:, :, 1, :],
            op0=ALU.mult, op1=ALU.mult)
        # clip and reorder to batch-major rows in one op: pc2[p, b, q] with
        # q = axis*2 + ih (r 8+b -> axis 0, r b -> axis 1)
        pc2 = mpool.tile([112, 8, 4], F32)
        pc2_view = bass.AP(pc2.tensor, pc2.offset,
                           [list(pc2.ap[0]), [1, 2], [2, 2], [4, 8]])
        pc_view = bass.AP(pc.tensor, pc.offset,
                          [list(pc.ap[0]), [16, 2], [-8, 2], [1, 8]])
        nc.vector.tensor_scalar(
            out=pc2_view, in0=bass.AP(pc.tensor, pc.offset + 8,
                                      [list(pc.ap[0]), [16, 2], [-8, 2], [1, 8]]),
            scalar1=0.0, scalar2=447.0, op0=ALU.max, op1=ALU.min)

        # coords to rows, already batch-major: pcT2[b*4 + q, p]; staged to
        # DRAM with a single contiguous DMA, then each batch broadcasts its
        # 448 coords with one stride-0 read
        pcT_ps = ps1.tile([32, 112], F32, tag="p1ps")
        nc.tensor.transpose(pcT_ps, pc2, ident[0:112, 0:112])
        pcT = mpool.tile([32, 112], F32)
        nc.scalar.copy(out=pcT, in_=pcT_ps)
        nc.sync.dma_start(
            out=bass.AP(pcd, 0, [[112, 32], [1, 112]]), in_=pcT)


        # ---------------- phase B: grid-sample ----------------
        wargs = {}

        def build_weights(b):
            # tent weights in [y, i] layout: w[p, yc, i] =
            # relu(1 - |c_i - (112*yc + p)|), c replicated across
            # partitions by gpsimd partition_broadcast. axis 0: wy
            # (coords row 8+b), axis 1: wx (row b). f32r, zero pads
            # to N=256 for the full-rate f32r moving path.
            # one stride-0 DRAM read replicates this batch's 448 coords
            # (i then j) onto every partition; pad lanes carry 1e9 so the
            # tents (and the f32r pad columns) are exactly zero there
            ycb = ypool.tile([112, 2, 256], F32, tag="ycb")
            nc.vector.memset(ycb[:, :, 224:256], 1e9)
            nc.sync.dma_start(
                out=ycb[:, :, 0:224],
                in_=bass.AP(pcd, b * 448, [[0, 112], [224, 2], [1, 224]]))
            # wsb pad columns [224:256) are never written: the matmuls
            # stream them into PSUM columns that no copy ever reads, so
            # their values are irrelevant (and f32r memset is ISA-illegal)
            wsb0 = wpool.tile([112, 4, 256], F32R, tag="w0")
            wsb1 = wpool.tile([112, 4, 256], F32R, tag="w1")
            # abs_max is not a valid hw TensorScalar op, and f32r memset
            # is illegal, so the tents are built NEGATED: u = |c - y| via
            # the Act Abs function, then min(u - 1, 0) = -relu(1 - u).
            # mm1 and mm2 both use negated weights; the signs cancel.
            for yc in range(4):
                u = ypool.tile([112, 2, 256], F32, tag="u")
                nc.scalar.activation(
                    out=u, in_=ycb, func=ACTF.Abs,
                    bias=nbcyc[:, yc:yc + 1], scale=1.0)
                nc.vector.tensor_scalar(
                    out=wsb0[:, yc, :], in0=u[:, 0, :], scalar1=1.0,
                    scalar2=0.0, op0=ALU.subtract, op1=ALU.min)
                nc.vector.tensor_scalar(
                    out=wsb1[:, yc, :], in0=u[:, 1, :], scalar1=1.0,
                    scalar2=0.0, op0=ALU.subtract, op1=ALU.min)
            wargs[(b, 0)] = wsb0
            wargs[(b, 1)] = wsb1

        def stage2(bt, wx, b, c):
            # x-axis sampling of the y-interpolated tile + output staging
            ob = psB.tile([112, 2, 256], F32, tag="ob")
            for ih in range(2):
                for xc in range(4):
                    nc.tensor.matmul(
                        ob[:, ih, :],
                        lhsT=bt[:, xc, ih * 112:(ih + 1) * 112],
                        rhs=wx[:, xc, :],
                        start=(xc == 0), stop=(xc == 3))
            osb = opool.tile([112, 2, SAM], F32, tag="osb")
            if c == 0:
                nc.vector.tensor_copy(out=osb, in_=ob[:, :, 0:224])
            else:
                nc.scalar.copy(out=osb, in_=ob[:, :, 0:224])
            nc.scalar.dma_start(
                out=out_dram[b, c].rearrange("(ih p) j -> p ih j", p=112),
                in_=osb)

        # software pipeline: weights one batch ahead; mm2 of tile n issues
        # after mm1 of tile n+1 so the PE never waits on the bt copies
        build_weights(0)
        pending = None
        for b in range(BSH):
            wy = wargs.pop((b, 0))
            wx = wargs.pop((b, 1))
            for c in range(3):
                if c == 0 and b + 1 < BSH:
                    build_weights(b + 1)
                at = dpool.tile([112, 4, IN], F32R, tag="at")
                dv = data_in[b, c].rearrange("(cc p) x -> p cc x", p=112)
                nc.sync.dma_start(out=at[:, 0:2, :], in_=dv[:, 0:2, :])
                nc.sync.dma_start(out=at[:, 2:4, :], in_=dv[:, 2:4, :])

                bt = btpool.tile([112, 4, SAM], F32R, tag="bt")
                for xc in range(4):
                    btp = psA.tile([112, 256], F32, tag="btp")
                    for yc in range(4):
                        nc.tensor.matmul(
                            btp,
                            lhsT=at[:, yc, xc * 112:(xc + 1) * 112],
                            rhs=wy[:, yc, :],
                            start=(yc == 0), stop=(yc == 3))
                    if xc % 2 == 0:
                        nc.scalar.copy(out=bt[:, xc, :], in_=btp[:, 0:224])
                    else:
                        nc.vector.tensor_copy(out=bt[:, xc, :], in_=btp[:, 0:224])
                if pending is not None:
                    stage2(*pending)
                pending = (bt, wx, b, c)
        stage2(*pending)
    nc.compile()
    return nc


def _static_consts(filter_w: np.ndarray):
    # fuse interp(448->224) + reflect-pad(->670) + 447-tap conv + P basis
    # into two [448, 224] matrices:  px = M1^T m,  pxP = MP^T m
    fw = filter_w.astype(np.float64)
    L = np.zeros((SAM, IN), dtype=np.float64)          # msn = L m
    j = np.arange(SAM)
    w = j / float(PAD)
    L[j, 2 * j] = 1.0 - w
    L[j, np.minimum(2 * j + 1, IN - 1)] += w
    S = np.zeros((GLOB, SAM), dtype=np.float64)        # sig = S msn
    S[np.arange(PAD), PAD - np.arange(PAD)] = 1.0      # left reflect
    S[PAD + np.arange(SAM), np.arange(SAM)] = 1.0      # center
    S[KSIZE + np.arange(PAD), PAD - 1 - np.arange(PAD)] = 1.0  # right reflect
    wm = np.zeros((GLOB, SAM), dtype=np.float64)       # conv: px = wm^T sig
    g = np.arange(GLOB)[:, None]
    o = np.arange(SAM)[None, :]
    k = g - o
    valid = (k >= 0) & (k < KSIZE)
    wm[valid] = fw[k[valid]]
    P = (np.arange(GLOB, dtype=np.float64) - PAD) / float(PAD)
    SL = S @ L                                         # [670, 448]
    M1 = SL.T @ wm                                     # [448, 224]
    MP = (P[:, None] * SL).T @ wm
    wcm = np.stack([M1, MP], axis=1).astype(np.float32)  # [448, 2, 224]
    nbcyc = -(np.arange(112, dtype=np.float32)[:, None]
              + 112.0 * np.arange(4, dtype=np.float32)[None, :])
    return {"wcm": wcm, "nbcyc": nbcyc}


def kernel(data: np.ndarray, structure_att: np.ndarray,
           filter_w: np.ndarray) -> np.ndarray:
    global last_results
    data = np.ascontiguousarray(data, dtype=np.float32)
    structure_att = np.ascontiguousarray(structure_att, dtype=np.float32)
    filter_w = np.ascontiguousarray(filter_w, dtype=np.float32)

    if "nc" not in _CACHE:
        _CACHE["nc"] = _build_program()
    nc = _CACHE["nc"]

    consts = _static_consts(filter_w)
    in_maps = []
    for core in range(NCORES):
        sl = slice(core * BSH, (core + 1) * BSH)
        in_maps.append({
            "data": data[sl], "att": structure_att[sl], **consts,
        })

    res = run_bass_kernel_spmd(nc, in_maps, core_ids=list(range(NCORES)))
    last_results = res
    out = np.concatenate([res.results[i]["out"] for i in range(NCORES)], axis=0)
    return out


# revision 6
# speedup vs baseline: 1.0428x; 1.0019x over previous
"""Trainium2 Bass kernel for nn_MASNET2 (structure-attention warped resampling).

Per batch:
  1. axis-max marginals of structure_att:
       y-profile: DVE free-dim max; x-profile: cc-fold (DVE+Pool) +
       gpsimd partition_all_reduce (profiles stay unnormalized -- the
       conv ratio xf = conv(P*m)/conv(m) is scale invariant).
  2. coords: interp(448->224) + reflect-pad + 447-tap conv + P basis all
     folded host-side into two [448, 224] matrices; on device 16 small
     fp32 matmuls map raw marginals -> per-partition grid coordinates.
  3. separable bilinear grid-sample as two all-f32r matmul stages
     (moving N=256 keeps the f32r full-rate path; weights self-load so
     the PE sequencer stays out of the way). Tent weight matrices are
     built directly in [y, i] layout: coords replicated across
     partitions with gpsimd partition_broadcast, |c-y| on DVE with a
     per-partition chunk-base bias, relu(1-u) on Act straight into the
     weight tile. No PE transposes, no PSUM staging.

Sharding: pure data-parallel, batch 64 -> 8 cores x 8.

DMA is the roofline (~87us of f32 traffic at 360 GB/s aggregate); DVE /
Act / Pool are balanced at ~46-50us each underneath it, and mm2 of
tile n issues after mm1 of tile n+1 so the PE never waits on the
PSUM->SBUF copies.
"""
import os
import sys

sys.path.insert(0, "/opt/trn_rl_repo")

import numpy as np
from contextlib import ExitStack

import concourse.bass as bass
import concourse.bacc as bacc
import concourse.tile as tile
from concourse import mybir, masks, bass_isa
from concourse.bass_utils import run_bass_kernel_spmd

F32 = mybir.dt.float32
F32R = mybir.dt.float32r
ALU = mybir.AluOpType
ACTF = mybir.ActivationFunctionType

SAM = 224
IN = 448
PAD = 223
GLOB = 670
KSIZE = 447
NCORES = 8
BSH = 8  # batch shard per core

_CACHE = {}

# expose the last run's results for test.py profiling
last_results = None


def _build_program():
    nc = bacc.Bacc("TRN2", num_devices=NCORES)

    data_in = nc.dram_tensor("data", (BSH, 3, IN, IN), F32R, kind="ExternalInput")
    att_in = nc.dram_tensor("att", (BSH, IN, IN), F32, kind="ExternalInput")
    wcm_in = nc.dram_tensor("wcm", (IN, 2, SAM), F32, kind="ExternalInput")
    nbcyc_in = nc.dram_tensor("nbcyc", (112, 4), F32, kind="ExternalInput")

    out_dram = nc.dram_tensor("out", (BSH, 3, SAM, SAM), F32, kind="ExternalOutput")
    pcd = nc.dram_tensor("pcd", (BSH, 4, 112), F32, kind="Internal")

    with tile.TileContext(nc) as tc, ExitStack() as ctx:
        consts = ctx.enter_context(tc.tile_pool(name="consts", bufs=1))
        apool = ctx.enter_context(tc.tile_pool(name="apool", bufs=5))
        dpool = ctx.enter_context(tc.tile_pool(name="dpool", bufs=10))
        fpool = ctx.enter_context(tc.tile_pool(name="fpool", bufs=2))
        arpool = ctx.enter_context(tc.tile_pool(name="arpool", bufs=2))
        mpool = ctx.enter_context(tc.tile_pool(name="mpool", bufs=1))
        ypool = ctx.enter_context(tc.tile_pool(name="ypool", bufs=4))
        wpool = ctx.enter_context(tc.tile_pool(name="wpool", bufs=2))
        btpool = ctx.enter_context(tc.tile_pool(name="btpool", bufs=4))
        opool = ctx.enter_context(tc.tile_pool(name="opool", bufs=3))
        ps1 = ctx.enter_context(tc.tile_pool(name="ps1", bufs=2, space="PSUM"))
        psA = ctx.enter_context(tc.tile_pool(name="psA", bufs=4, space="PSUM"))
        psB = ctx.enter_context(tc.tile_pool(name="psB", bufs=2, space="PSUM"))

        # small consts first on the scalar queue: dispatched at t=0,
        # transfers trivial, so the att loads own the DMA engines
        nbcyc = consts.tile([112, 4], F32)
        nc.scalar.dma_start(out=nbcyc, in_=nbcyc_in[:, :])
        ident = consts.tile([128, 128], F32)
        masks.make_identity(nc, ident[:])

        # ---------------- phase 1: marginals for all batches ----------------
        # marg64[p, cc, b] = y-profile value at y = cc*112+p  (max over x)
        # marg16x row b    = x-profile of batch b (replicated via all-reduce)
        marg64 = mpool.tile([112, 4, BSH], F32)
        margT = mpool.tile([112, 4, 16], F32)
        for b in range(BSH):
            att_t = apool.tile([112, 4, IN], F32, tag="att_t")
            nc.sync.dma_start(
                out=att_t, in_=att_in[b].rearrange("(cc p) x -> p cc x", p=112))
            # y-profile: max over x (free dim) on DVE
            nc.vector.tensor_reduce(
                out=marg64[:, :, b], in_=att_t, axis=mybir.AxisListType.X,
                op=ALU.max)
            # x-profile (max over all 448 y): the real Pool engine has no
            # elementwise ops, only ucode (all-reduce / affine_select), so
            # the cc-fold runs either as DVE maxes (path A, short chain:
            # used for the last batches on the critical tail) or entirely
            # off-DVE as a double all-reduce (path B): AR#1 replicates the
            # per-(cc,x) column max to every partition, mod-32-aligned Act
            # copies stack the four cc slices into four 32-partition
            # bands, AR#2 folds them.
            if b >= 6:
                f1 = fpool.tile([128, IN], F32, tag="f1")
                nc.vector.tensor_tensor(
                    out=f1[0:112, :], in0=att_t[:, 0, :], in1=att_t[:, 1, :],
                    op=ALU.max)
                f2 = fpool.tile([112, IN], F32, tag="f2")
                nc.vector.tensor_tensor(
                    out=f2, in0=att_t[:, 2, :], in1=att_t[:, 3, :], op=ALU.max)
                nc.vector.tensor_tensor(
                    out=f1[0:112, :], in0=f1[0:112, :], in1=f2, op=ALU.max)
            else:
                ar1 = fpool.tile([112, 4, IN], F32, tag="ar1")
                nc.gpsimd.partition_all_reduce(
                    out_ap=ar1, in_ap=att_t, channels=112,
                    reduce_op=bass_isa.ReduceOp.max)
                # ar1 is replicated: partitions 0:32 hold every cc slice
                f1 = fpool.tile([128, IN], F32, tag="f1")
                for cc in range(4):
                    nc.scalar.copy(
                        out=f1[32 * cc:32 * cc + 32, :],
                        in_=ar1[0:32, cc, :])
            arx = arpool.tile([112, IN], F32, tag="arx")
            nc.gpsimd.partition_all_reduce(
                out_ap=arx, in_ap=f1[0:112, :], channels=112,
                reduce_op=bass_isa.ReduceOp.max)
            # transpose the replicated profile so x lands on partitions,
            # then one strided copy of column 0 fills margT[:, :, b]
            atr = ps1.tile([112, 4, 112], F32, tag="p1ps")
            for xc in range(4):
                nc.tensor.transpose(
                    atr[:, xc, :], arx[:, xc * 112:(xc + 1) * 112],
                    ident[0:112, 0:112])
            nc.scalar.copy(out=margT[:, :, b:b + 1], in_=atr[:, :, 0:1])

        # the big fused-conv const rides the sync queue between att and data
        wcm = consts.tile([112, 4, 2, SAM], F32)
        nc.sync.dma_start(
            out=wcm, in_=wcm_in.rearrange("(xc p) s o -> p xc s o", p=112))

        # ---------------- coords: one fused linear map + ratio ----------
        # margT[p, xc, r]: marginal value at x = xc*112+p for row r
        # (r 0..7: x-profiles -> j coords; r 8..15: y-profiles -> i coords)
        nc.vector.tensor_copy(out=margT[:, :, 8:16], in_=marg64)

        # px_ps[p, oh, 0, r] = conv(m)[oh*112+p], [.., 1, r] = conv(P*m)
        px_ps = ps1.tile([112, 2, 2, 16], F32, tag="p1ps")
        for s in range(2):
            for oh in range(2):
                for xc in range(4):
                    nc.tensor.matmul(
                        px_ps[:, oh, s, :],
                        lhsT=wcm[:, xc, s, oh * 112:(oh + 1) * 112],
                        rhs=margT[:, xc, :],
                        start=(xc == 0), stop=(xc == 3))
        pxs = mpool.tile([112, 2, 2, 16], F32)
        nc.scalar.copy(out=pxs, in_=px_ps)
        rec = mpool.tile([112, 2, 16], F32)
        nc.vector.reciprocal(out=rec, in_=pxs[:, :, 0, :])
        pc = mpool.tile([112, 2, 16], F32)
        nc.vector.scalar_tensor_tensor(
            out=pc, in0=rec, scalar=447.0, in1=pxs[:, :, 1, :],
            op0=ALU.mult, op1=ALU.mult)
        # clip and reorder to batch-major rows in one op: pc2[p, b, q] with
        # q = axis*2 + ih (r 8+b -> axis 0, r b -> axis 1)
        pc2 = mpool.tile([112, 8, 4], F32)
        pc2_view = bass.AP(pc2.tensor, pc2.offset,
                           [list(pc2.ap[0]), [1, 2], [2, 2], [4, 8]])
        pc_view = bass.AP(pc.tensor, pc.offset,
                          [list(pc.ap[0]), [16, 2], [-8, 2], [1, 8]])
        nc.vector.tensor_scalar(
            out=pc2_view, in0=bass.AP(pc.tensor, pc.offset + 8,
                                      [list(pc.ap[0]), [16, 2], [-8, 2], [1, 8]]),
            scalar1=0.0, scalar2=447.0, op0=ALU.max, op1=ALU.min)

        # coords to rows, already batch-major: pcT2[b*4 + q, p]; staged to
        # DRAM with a single contiguous DMA, then each batch broadcasts its
        # 448 coords with one stride-0 read
        pcT_ps = ps1.tile([32, 112], F32, tag="p1ps")
        nc.tensor.transpose(pcT_ps, pc2, ident[0:112, 0:112])
        pcT = mpool.tile([32, 112], F32)
        nc.scalar.copy(out=pcT, in_=pcT_ps)
        nc.sync.dma_start(
            out=bass.AP(pcd, 0, [[112, 32], [1, 112]]), in_=pcT)


        # ---------------- phase B: grid-sample ----------------
        wargs = {}

        def build_weights(b):
            # tent weights in [y, i] layout: w[p, yc, i] =
            # relu(1 - |c_i - (112*yc + p)|), c replicated across
            # partitions by gpsimd partition_broadcast. axis 0: wy
            # (coords row 8+b), axis 1: wx (row b). f32r, zero pads
            # to N=256 for the full-rate f32r moving path.
            # one stride-0 DRAM read replicates this batch's 448 coords
            # (i then j) onto every partition; pad lanes carry 1e9 so the
            # tents (and the f32r pad columns) are exactly zero there
            ycb = ypool.tile([112, 2, 256], F32, tag="ycb")
            nc.vector.memset(ycb[:, :, 224:256], 1e9)
            nc.sync.dma_start(
                out=ycb[:, :, 0:224],
                in_=bass.AP(pcd, b * 448, [[0, 112], [224, 2], [1, 224]]))
            # wsb pad columns [224:256) are never written: the matmuls
            # stream them into PSUM columns that no copy ever reads, so
            # their values are irrelevant (and f32r memset is ISA-illegal)
            wsb0 = wpool.tile([112, 4, 256], F32R, tag="w0")
            wsb1 = wpool.tile([112, 4, 256], F32R, tag="w1")
            # abs_max is not a valid hw TensorScalar op, and f32r memset
            # is illegal, so the tents are built NEGATED: u = |c - y| via
            # the Act Abs function, then min(u - 1, 0) = -relu(1 - u).
            # mm1 and mm2 both use negated weights; the signs cancel.
            for yc in range(4):
                u = ypool.tile([112, 2, 256], F32, tag="u")
                nc.scalar.activation(
                    out=u, in_=ycb, func=ACTF.Abs,
                    bias=nbcyc[:, yc:yc + 1], scale=1.0)
                nc.vector.tensor_scalar(
                    out=wsb0[:, yc, :], in0=u[:, 0, :], scalar1=1.0,
                    scalar2=0.0, op0=ALU.subtract, op1=ALU.min)
                nc.vector.tensor_scalar(
                    out=wsb1[:, yc, :], in0=u[:, 1, :], scalar1=1.0,
                    scalar2=0.0, op0=ALU.subtract, op1=ALU.min)
            wargs[(b, 0)] = wsb0
            wargs[(b, 1)] = wsb1

        def stage2(bt, wx, b, c):
            # x-axis sampling of the y-interpolated tile + output staging
            ob = psB.tile([112, 2, 256], F32, tag="ob")
            for ih in range(2):
                for xc in range(4):
                    nc.tensor.matmul(
                        ob[:, ih, :],
                        lhsT=bt[:, xc, ih * 112:(ih + 1) * 112],
                        rhs=wx[:, xc, :],
                        start=(xc == 0), stop=(xc == 3))
            osb = opool.tile([112, 2, SAM], F32, tag="osb")
            if c == 0:
                nc.vector.tensor_copy(out=osb, in_=ob[:, :, 0:224])
            else:
                nc.scalar.copy(out=osb, in_=ob[:, :, 0:224])
            nc.scalar.dma_start(
                out=out_dram[b, c].rearrange("(ih p) j -> p ih j", p=112),
                in_=osb)

        # software pipeline: weights one batch ahead; mm2 of tile n issues
        # after mm1 of tile n+1 so the PE never waits on the bt copies
        build_weights(0)
        pending = None
        for b in range(BSH):
            wy = wargs.pop((b, 0))
            wx = wargs.pop((b, 1))
            for c in range(3):
                if c == 0 and b + 1 < BSH:
                    build_weights(b + 1)
                at = dpool.tile([112, 4, IN], F32R, tag="at")
                dv = data_in[b, c].rearrange("(cc p) x -> p cc x", p=112)
                nc.sync.dma_start(out=at[:, 0:2, :], in_=dv[:, 0:2, :])
                nc.sync.dma_start(out=at[:, 2:4, :], in_=dv[:, 2:4, :])

                bt = btpool.tile([112, 4, SAM], F32R, tag="bt")
                for xc in range(4):
                    btp = psA.tile([112, 256], F32, tag="btp")
                    for yc in range(4):
                        nc.tensor.matmul(
                            btp,
                            lhsT=at[:, yc, xc * 112:(xc + 1) * 112],
                            rhs=wy[:, yc, :],
                            start=(yc == 0), stop=(yc == 3))
                    if xc % 2 == 0:
                        nc.scalar.copy(out=bt[:, xc, :], in_=btp[:, 0:224])
                    else:
                        nc.vector.tensor_copy(out=bt[:, xc, :], in_=btp[:, 0:224])
                if pending is not None:
                    stage2(*pending)
                pending = (bt, wx, b, c)
        stage2(*pending)
    nc.compile()
    return nc


def _static_consts(filter_w: np.ndarray):
    # fuse interp(448->224) + reflect-pad(->670) + 447-tap conv + P basis
    # into two [448, 224] matrices:  px = M1^T m,  pxP = MP^T m
    fw = filter_w.astype(np.float64)
    L = np.zeros((SAM, IN), dtype=np.float64)          # msn = L m
    j = np.arange(SAM)
    w = j / float(PAD)
    L[j, 2 * j] = 1.0 - w
    L[j, np.minimum(2 * j + 1, IN - 1)] += w
    S = np.zeros((GLOB, SAM), dtype=np.float64)        # sig = S msn
    S[np.arange(PAD), PAD - np.arange(PAD)] = 1.0      # left reflect
    S[PAD + np.arange(SAM), np.arange(SAM)] = 1.0      # center
    S[KSIZE + np.arange(PAD), PAD - 1 - np.arange(PAD)] = 1.0  # right reflect
    wm = np.zeros((GLOB, SAM), dtype=np.float64)       # conv: px = wm^T sig
    g = np.arange(GLOB)[:, None]
    o = np.arange(SAM)[None, :]
    k = g - o
    valid = (k >= 0) & (k < KSIZE)
    wm[valid] = fw[k[valid]]
    P = (np.arange(GLOB, dtype=np.float64) - PAD) / float(PAD)
    SL = S @ L                                         # [670, 448]
    M1 = SL.T @ wm                                     # [448, 224]
    MP = (P[:, None] * SL).T @ wm
    wcm = np.stack([M1, MP], axis=1).astype(np.float32)  # [448, 2, 224]
    nbcyc = -(np.arange(112, dtype=np.float32)[:, None]
              + 112.0 * np.arange(4, dtype=np.float32)[None, :])
    return {"wcm": wcm, "nbcyc": nbcyc}


def kernel(data: np.ndarray, structure_att: np.ndarray,
           filter_w: np.ndarray) -> np.ndarray:
    global last_results
    data = np.ascontiguousarray(data, dtype=np.float32)
    structure_att = np.ascontiguousarray(structure_att, dtype=np.float32)
    filter_w = np.ascontiguousarray(filter_w, dtype=np.float32)

    if "nc" not in _CACHE:
        _CACHE["nc"] = _build_program()
    nc = _CACHE["nc"]

    consts = _static_consts(filter_w)
    in_maps = []
    for core in range(NCORES):
        sl = slice(core * BSH, (core + 1) * BSH)
        in_maps.append({
            "data": data[sl], "att": structure_att[sl], **consts,
        })

    res = run_bass_kernel_spmd(nc, in_maps, core_ids=list(range(NCORES)))
    last_results = res
    out = np.concatenate([res.results[i]["out"] for i in range(NCORES)], axis=0)
    return out


# revision 7
# speedup vs baseline: 1.0435x; 1.0007x over previous
"""Trainium2 Bass kernel for nn_MASNET2 (structure-attention warped resampling).

Per batch:
  1. axis-max marginals of structure_att:
       y-profile: DVE free-dim max; x-profile: cc-fold (DVE+Pool) +
       gpsimd partition_all_reduce (profiles stay unnormalized -- the
       conv ratio xf = conv(P*m)/conv(m) is scale invariant).
  2. coords: interp(448->224) + reflect-pad + 447-tap conv + P basis all
     folded host-side into two [448, 224] matrices; on device 16 small
     fp32 matmuls map raw marginals -> per-partition grid coordinates.
  3. separable bilinear grid-sample as two all-f32r matmul stages
     (moving N=256 keeps the f32r full-rate path; weights self-load so
     the PE sequencer stays out of the way). Tent weight matrices are
     built directly in [y, i] layout: coords replicated across
     partitions with gpsimd partition_broadcast, |c-y| on DVE with a
     per-partition chunk-base bias, relu(1-u) on Act straight into the
     weight tile. No PE transposes, no PSUM staging.

Sharding: pure data-parallel, batch 64 -> 8 cores x 8.

DMA is the roofline (~87us of f32 traffic at 360 GB/s aggregate); DVE /
Act / Pool are balanced at ~46-50us each underneath it, and mm2 of
tile n issues after mm1 of tile n+1 so the PE never waits on the
PSUM->SBUF copies.
"""
import os
import sys

sys.path.insert(0, "/opt/trn_rl_repo")

import numpy as np
from contextlib import ExitStack

import concourse.bass as bass
import concourse.bacc as bacc
import concourse.tile as tile
from concourse import mybir, masks, bass_isa
from concourse.bass_utils import run_bass_kernel_spmd

F32 = mybir.dt.float32
F32R = mybir.dt.float32r
ALU = mybir.AluOpType
ACTF = mybir.ActivationFunctionType

SAM = 224
IN = 448
PAD = 223
GLOB = 670
KSIZE = 447
NCORES = 8
BSH = 8  # batch shard per core

_CACHE = {}

# expose the last run's results for test.py profiling
last_results = None


def _build_program():
    nc = bacc.Bacc("TRN2", num_devices=NCORES)

    data_in = nc.dram_tensor("data", (BSH, 3, IN, IN), F32R, kind="ExternalInput")
    att_in = nc.dram_tensor("att", (BSH, IN, IN), F32, kind="ExternalInput")
    wcm_in = nc.dram_tensor("wcm", (IN, 2, SAM), F32, kind="ExternalInput")
    nbcyc_in = nc.dram_tensor("nbcyc", (112, 4), F32, kind="ExternalInput")

    out_dram = nc.dram_tensor("out", (BSH, 3, SAM, SAM), F32, kind="ExternalOutput")
    pcd = nc.dram_tensor("pcd", (BSH, 4, 112), F32, kind="Internal")

    with tile.TileContext(nc) as tc, ExitStack() as ctx:
        consts = ctx.enter_context(tc.tile_pool(name="consts", bufs=1))
        apool = ctx.enter_context(tc.tile_pool(name="apool", bufs=5))
        dpool = ctx.enter_context(tc.tile_pool(name="dpool", bufs=10))
        fpool = ctx.enter_context(tc.tile_pool(name="fpool", bufs=2))
        arpool = ctx.enter_context(tc.tile_pool(name="arpool", bufs=2))
        mpool = ctx.enter_context(tc.tile_pool(name="mpool", bufs=1))
        ypool = ctx.enter_context(tc.tile_pool(name="ypool", bufs=4))
        wpool = ctx.enter_context(tc.tile_pool(name="wpool", bufs=2))
        btpool = ctx.enter_context(tc.tile_pool(name="btpool", bufs=4))
        opool = ctx.enter_context(tc.tile_pool(name="opool", bufs=3))
        ps1 = ctx.enter_context(tc.tile_pool(name="ps1", bufs=2, space="PSUM"))
        psA = ctx.enter_context(tc.tile_pool(name="psA", bufs=4, space="PSUM"))
        psB = ctx.enter_context(tc.tile_pool(name="psB", bufs=2, space="PSUM"))

        # small consts first on the scalar queue: dispatched at t=0,
        # transfers trivial, so the att loads own the DMA engines
        nbcyc = consts.tile([112, 4], F32)
        nc.scalar.dma_start(out=nbcyc, in_=nbcyc_in[:, :])
        ident = consts.tile([128, 128], F32)
        masks.make_identity(nc, ident[:])

        # ---------------- phase 1: marginals for all batches ----------------
        # marg64[p, cc, b] = y-profile value at y = cc*112+p  (max over x)
        # marg16x row b    = x-profile of batch b (replicated via all-reduce)
        marg64 = mpool.tile([112, 4, BSH], F32)
        margT = mpool.tile([112, 4, 16], F32)
        for b in range(BSH):
            att_t = apool.tile([112, 4, IN], F32, tag="att_t")
            nc.sync.dma_start(
                out=att_t, in_=att_in[b].rearrange("(cc p) x -> p cc x", p=112))
            # y-profile: max over x (free dim) on DVE
            nc.vector.tensor_reduce(
                out=marg64[:, :, b], in_=att_t, axis=mybir.AxisListType.X,
                op=ALU.max)
            # x-profile (max over all 448 y): the real Pool engine has no
            # elementwise ops, only ucode (all-reduce / affine_select), so
            # the cc-fold runs either as DVE maxes (path A, short chain:
            # used for the last batches on the critical tail) or entirely
            # off-DVE as a double all-reduce (path B): AR#1 replicates the
            # per-(cc,x) column max to every partition, mod-32-aligned Act
            # copies stack the four cc slices into four 32-partition
            # bands, AR#2 folds them.
            if b >= 6:
                f1 = fpool.tile([128, IN], F32, tag="f1")
                nc.vector.tensor_tensor(
                    out=f1[0:112, :], in0=att_t[:, 0, :], in1=att_t[:, 1, :],
                    op=ALU.max)
                f2 = fpool.tile([112, IN], F32, tag="f2")
                nc.vector.tensor_tensor(
                    out=f2, in0=att_t[:, 2, :], in1=att_t[:, 3, :], op=ALU.max)
                nc.vector.tensor_tensor(
                    out=f1[0:112, :], in0=f1[0:112, :], in1=f2, op=ALU.max)
            else:
                ar1 = fpool.tile([112, 4, IN], F32, tag="ar1")
                nc.gpsimd.partition_all_reduce(
                    out_ap=ar1, in_ap=att_t, channels=112,
                    reduce_op=bass_isa.ReduceOp.max)
                # ar1 is replicated: partitions 0:32 hold every cc slice
                f1 = fpool.tile([128, IN], F32, tag="f1")
                for cc in range(4):
                    nc.scalar.copy(
                        out=f1[32 * cc:32 * cc + 32, :],
                        in_=ar1[0:32, cc, :])
            arx = arpool.tile([112, IN], F32, tag="arx")
            nc.gpsimd.partition_all_reduce(
                out_ap=arx, in_ap=f1[0:112, :], channels=112,
                reduce_op=bass_isa.ReduceOp.max)
            # transpose the replicated profile so x lands on partitions,
            # then one strided copy of column 0 fills margT[:, :, b]
            atr = ps1.tile([112, 4, 112], F32, tag="p1ps")
            for xc in range(4):
                nc.tensor.transpose(
                    atr[:, xc, :], arx[:, xc * 112:(xc + 1) * 112],
                    ident[0:112, 0:112])
            nc.scalar.copy(out=margT[:, :, b:b + 1], in_=atr[:, :, 0:1])

        # the big fused-conv const rides the sync queue between att and data
        wcm = consts.tile([112, 4, 2, SAM], F32)
        nc.sync.dma_start(
            out=wcm, in_=wcm_in.rearrange("(xc p) s o -> p xc s o", p=112))

        # ---------------- coords: one fused linear map + ratio ----------
        # margT[p, xc, r]: marginal value at x = xc*112+p for row r
        # (r 0..7: x-profiles -> j coords; r 8..15: y-profiles -> i coords)
        nc.vector.tensor_copy(out=margT[:, :, 8:16], in_=marg64)

        # px_ps[p, oh, 0, r] = conv(m)[oh*112+p], [.., 1, r] = conv(P*m)
        px_ps = ps1.tile([112, 2, 2, 16], F32, tag="p1ps")
        for s in range(2):
            for oh in range(2):
                for xc in range(4):
                    nc.tensor.matmul(
                        px_ps[:, oh, s, :],
                        lhsT=wcm[:, xc, s, oh * 112:(oh + 1) * 112],
                        rhs=margT[:, xc, :],
                        start=(xc == 0), stop=(xc == 3))
        pxs = mpool.tile([112, 2, 2, 16], F32)
        nc.scalar.copy(out=pxs, in_=px_ps)
        rec = mpool.tile([112, 2, 16], F32)
        nc.vector.reciprocal(out=rec, in_=pxs[:, :, 0, :])
        pc = mpool.tile([112, 2, 16], F32)
        nc.vector.scalar_tensor_tensor(
            out=pc, in0=rec, scalar=447.0, in1=pxs[:, :, 1, :],
            op0=ALU.mult, op1=ALU.mult)
        # clip and reorder to batch-major rows in one op: pc2[p, b, q] with
        # q = axis*2 + ih (r 8+b -> axis 0, r b -> axis 1)
        pc2 = mpool.tile([112, 8, 4], F32)
        pc2_view = bass.AP(pc2.tensor, pc2.offset,
                           [list(pc2.ap[0]), [1, 2], [2, 2], [4, 8]])
        pc_view = bass.AP(pc.tensor, pc.offset,
                          [list(pc.ap[0]), [16, 2], [-8, 2], [1, 8]])
        nc.vector.tensor_scalar(
            out=pc2_view, in0=bass.AP(pc.tensor, pc.offset + 8,
                                      [list(pc.ap[0]), [16, 2], [-8, 2], [1, 8]]),
            scalar1=0.0, scalar2=447.0, op0=ALU.max, op1=ALU.min)

        # coords to rows, already batch-major: pcT2[b*4 + q, p]; staged to
        # DRAM with a single contiguous DMA, then each batch broadcasts its
        # 448 coords with one stride-0 read
        pcT_ps = ps1.tile([32, 112], F32, tag="p1ps")
        nc.tensor.transpose(pcT_ps, pc2, ident[0:112, 0:112])
        pcT = mpool.tile([32, 112], F32)
        nc.scalar.copy(out=pcT, in_=pcT_ps)
        nc.sync.dma_start(
            out=bass.AP(pcd, 0, [[112, 32], [1, 112]]), in_=pcT)


        # ---------------- phase B: grid-sample ----------------
        wargs = {}

        def build_weights(b):
            # tent weights in [y, i] layout: w[p, yc, i] =
            # relu(1 - |c_i - (112*yc + p)|), c replicated across
            # partitions by gpsimd partition_broadcast. axis 0: wy
            # (coords row 8+b), axis 1: wx (row b). f32r, zero pads
            # to N=256 for the full-rate f32r moving path.
            # one stride-0 DRAM read replicates this batch's 448 coords
            # (i then j) onto every partition; pad lanes carry 1e9 so the
            # tents (and the f32r pad columns) are exactly zero there
            ycb = ypool.tile([112, 2, 256], F32, tag="ycb")
            nc.vector.memset(ycb[:, :, 224:256], 1e9)
            nc.sync.dma_start(
                out=ycb[:, :, 0:224],
                in_=bass.AP(pcd, b * 448, [[0, 112], [224, 2], [1, 224]]))
            # wsb pad columns [224:256) are never written: the matmuls
            # stream them into PSUM columns that no copy ever reads, so
            # their values are irrelevant (and f32r memset is ISA-illegal)
            wsb0 = wpool.tile([112, 4, 256], F32R, tag="w0")
            wsb1 = wpool.tile([112, 4, 256], F32R, tag="w1")
            # abs_max is not a valid hw TensorScalar op, and f32r memset
            # is illegal, so the tents are built NEGATED: u = |c - y| via
            # the Act Abs function, then min(u - 1, 0) = -relu(1 - u).
            # mm1 and mm2 both use negated weights; the signs cancel.
            for yc in range(4):
                u = ypool.tile([112, 2, 256], F32, tag="u")
                nc.scalar.activation(
                    out=u, in_=ycb, func=ACTF.Abs,
                    bias=nbcyc[:, yc:yc + 1], scale=1.0)
                nc.vector.tensor_scalar(
                    out=wsb0[:, yc, :], in0=u[:, 0, :], scalar1=1.0,
                    scalar2=0.0, op0=ALU.subtract, op1=ALU.min)
                nc.vector.tensor_scalar(
                    out=wsb1[:, yc, :], in0=u[:, 1, :], scalar1=1.0,
                    scalar2=0.0, op0=ALU.subtract, op1=ALU.min)
            wargs[(b, 0)] = wsb0
            wargs[(b, 1)] = wsb1

        def stage2(bt, wx, b, c):
            # x-axis sampling of the y-interpolated tile + output staging
            ob = psB.tile([112, 2, 256], F32, tag="ob")
            for ih in range(2):
                for xc in range(4):
                    nc.tensor.matmul(
                        ob[:, ih, :],
                        lhsT=bt[:, xc, ih * 112:(ih + 1) * 112],
                        rhs=wx[:, xc, :],
                        start=(xc == 0), stop=(xc == 3))
            osb = opool.tile([112, 2, SAM], F32, tag="osb")
            if c == 0:
                nc.vector.tensor_copy(out=osb, in_=ob[:, :, 0:224])
            else:
                nc.scalar.copy(out=osb, in_=ob[:, :, 0:224])
            nc.scalar.dma_start(
                out=out_dram[b, c].rearrange("(ih p) j -> p ih j", p=112),
                in_=osb)

        # software pipeline: weights one batch ahead; mm2 of tile n issues
        # after mm1 of tile n+1 so the PE never waits on the bt copies
        build_weights(0)
        pending = None
        for b in range(BSH):
            wy = wargs.pop((b, 0))
            wx = wargs.pop((b, 1))
            for c in range(3):
                if c == 0 and b + 1 < BSH:
                    build_weights(b + 1)
                at = dpool.tile([112, 4, IN], F32R, tag="at")
                dv = data_in[b, c].rearrange("(cc p) x -> p cc x", p=112)
                nc.sync.dma_start(out=at[:, 0:2, :], in_=dv[:, 0:2, :])
                nc.sync.dma_start(out=at[:, 2:4, :], in_=dv[:, 2:4, :])

                bt = btpool.tile([112, 4, SAM], F32R, tag="bt")
                for xc in range(4):
                    btp = psA.tile([112, 256], F32, tag="btp")
                    for yc in range(4):
                        nc.tensor.matmul(
                            btp,
                            lhsT=at[:, yc, xc * 112:(xc + 1) * 112],
                            rhs=wy[:, yc, :],
                            start=(yc == 0), stop=(yc == 3))
                    if xc % 2 == 0:
                        nc.scalar.copy(out=bt[:, xc, :], in_=btp[:, 0:224])
                    else:
                        nc.vector.tensor_copy(out=bt[:, xc, :], in_=btp[:, 0:224])
                if pending is not None:
                    stage2(*pending)
                if b == BSH - 1:
                    # last batch: no deferral, so the final mm2s overlap
                    # the remaining mm1s instead of trailing them
                    stage2(bt, wx, b, c)
                    pending = None
                else:
                    pending = (bt, wx, b, c)
        if pending is not None:
            stage2(*pending)
    nc.compile()
    return nc


def _static_consts(filter_w: np.ndarray):
    # fuse interp(448->224) + reflect-pad(->670) + 447-tap conv + P basis
    # into two [448, 224] matrices:  px = M1^T m,  pxP = MP^T m
    fw = filter_w.astype(np.float64)
    L = np.zeros((SAM, IN), dtype=np.float64)          # msn = L m
    j = np.arange(SAM)
    w = j / float(PAD)
    L[j, 2 * j] = 1.0 - w
    L[j, np.minimum(2 * j + 1, IN - 1)] += w
    S = np.zeros((GLOB, SAM), dtype=np.float64)        # sig = S msn
    S[np.arange(PAD), PAD - np.arange(PAD)] = 1.0      # left reflect
    S[PAD + np.arange(SAM), np.arange(SAM)] = 1.0      # center
    S[KSIZE + np.arange(PAD), PAD - 1 - np.arange(PAD)] = 1.0  # right reflect
    wm = np.zeros((GLOB, SAM), dtype=np.float64)       # conv: px = wm^T sig
    g = np.arange(GLOB)[:, None]
    o = np.arange(SAM)[None, :]
    k = g - o
    valid = (k >= 0) & (k < KSIZE)
    wm[valid] = fw[k[valid]]
    P = (np.arange(GLOB, dtype=np.float64) - PAD) / float(PAD)
    SL = S @ L                                         # [670, 448]
    M1 = SL.T @ wm                                     # [448, 224]
    MP = (P[:, None] * SL).T @ wm
    wcm = np.stack([M1, MP], axis=1).astype(np.float32)  # [448, 2, 224]
    nbcyc = -(np.arange(112, dtype=np.float32)[:, None]
              + 112.0 * np.arange(4, dtype=np.float32)[None, :])
    return {"wcm": wcm, "nbcyc": nbcyc}


def kernel(data: np.ndarray, structure_att: np.ndarray,
           filter_w: np.ndarray) -> np.ndarray:
    global last_results
    data = np.ascontiguousarray(data, dtype=np.float32)
    structure_att = np.ascontiguousarray(structure_att, dtype=np.float32)
    filter_w = np.ascontiguousarray(filter_w, dtype=np.float32)

    if "nc" not in _CACHE:
        _CACHE["nc"] = _build_program()
    nc = _CACHE["nc"]

    consts = _static_consts(filter_w)
    in_maps = []
    for core in range(NCORES):
        sl = slice(core * BSH, (core + 1) * BSH)
        in_maps.append({
            "data": data[sl], "att": structure_att[sl], **consts,
        })

    res = run_bass_kernel_spmd(nc, in_maps, core_ids=list(range(NCORES)))
    last_results = res
    out = np.concatenate([res.results[i]["out"] for i in range(NCORES)], axis=0)
    return out
